# revision 25
# baseline (speedup 1.0000x reference)
"""MoE layer (E=8 experts, top-2 routing) on 8 trn2 NeuronCores.

Strategy
--------
The reference computes the FFN densely (every token through every expert)
and then combines with a (T, E) weight matrix that is zero outside each
token's top-2 experts.  Only the sparse grouped-GEMM is needed:

  host   : router (tiny fp32 GEMM + softmax + top-2), group tokens by
           expert, pad each group to C tokens, transpose to (D, C).
  device : core e runs expert e's MLP on its token group:
              OUT^T = W2^T-contract( silu(W1^T-contract(X^T) + b1) ) + b2
           in transposed-activation layout (contraction dim on SBUF
           partitions) so no on-device transposes are needed.  bf16
           operands, fp32 PSUM accumulation.  Weights/activations are
           pre-packed on host so every DMA is one large fully-contiguous
           transfer in device consumption order.
  host   : scale rows by the gate weights and scatter-add back; aux loss
           computed exactly on host.
"""

import os
import numpy as np
import ml_dtypes
from contextlib import ExitStack

import concourse.bass as bass
import concourse.tile as tile
from concourse import bacc, mybir
from concourse.bass_utils import run_bass_kernel_spmd

BF16 = ml_dtypes.bfloat16

B, S, D, H, E, TOP_K = 2, 2048, 1024, 4096, 8, 2
T = B * S
P = 128
DT, HT = D // P, H // P       # 8 d-tiles, 32 h-tiles
WG = 512                      # w1 column-group width (4 h-tiles per group)
NG = H // WG                  # 8 w1 groups
NQ = 4                        # w2 quarters
HQ = HT // NQ                 # 8 h-tiles per w2 quarter

_cache: dict = {}
last_results = None           # BassKernelResults of the most recent run (for test.py)


def _plan(maxcnt: int):
    """Pick padded token count C and pass split (tokens processed per pass).

    PASS <= 512 so each (h,d) step is a single matmul into one PSUM bank;
    npass minimal so per-instruction overhead is minimal."""
    c = max(2, (maxcnt + 1) // 2 * 2)
    npass = -(-c // 512)
    ps = -(-c // npass)
    ps = (ps + 1) // 2 * 2
    return ps * npass, npass, ps


def _chunks(ps: int):
    """(sbuf_off, psum_off, width) sub-chunks of one pass.

    Two balanced chunks (each >128 cols so one MM's stream covers the
    previous MM's PE drain); second chunk starts at PSUM column 512 so each
    matmul output stays inside a single 2KB PSUM bank."""
    if ps <= 512:
        return [(0, 0, ps)]
    ca = (ps + 1) // 2
    return [(0, 0, ca), (ca, 512, ps - ca)]


def _xpad(ps: int):
    """xt row stride (elements): pass width padded to a 64-element multiple
    so each d-stripe's slice starts 128B-aligned in SBUF."""
    return (ps + 63) // 64 * 64


def _build(C: int, npass: int, PASS: int):
    """Build + compile the SPMD per-expert MLP program."""
    XP = _xpad(PASS)
    nc = bacc.Bacc("TRN2", target_bir_lowering=False, debug=False, num_devices=E)
    xt = nc.dram_tensor("xt", [npass, P, DT * XP], mybir.dt.bfloat16, kind="ExternalInput").ap()
    w1 = nc.dram_tensor("w1", [NG, P, DT * WG], mybir.dt.bfloat16, kind="ExternalInput").ap()
    bias = nc.dram_tensor("bias", [P, HT + DT], mybir.dt.float32, kind="ExternalInput").ap()
    w2 = nc.dram_tensor("w2", [NQ, P, HQ * D], mybir.dt.bfloat16, kind="ExternalInput").ap()
    ot = nc.dram_tensor("outt", [D, C], mybir.dt.float32, kind="ExternalOutput").ap()
    ot_r = ot.rearrange("(a p) n -> a p n", p=P)

    cks = _chunks(PASS)
    PB = 1024 if len(cks) > 1 else 512   # psum tile width (1 or 2 banks)

    with tile.TileContext(nc) as tc, ExitStack() as ctx:
        const = ctx.enter_context(tc.tile_pool(name="const", bufs=1))
        htp = ctx.enter_context(tc.tile_pool(name="ht", bufs=HT))
        outp = ctx.enter_context(tc.tile_pool(name="out", bufs=2))
        p1 = ctx.enter_context(tc.tile_pool(name="p1", bufs=6 if PB == 512 else 2, space="PSUM"))
        p2 = ctx.enter_context(tc.tile_pool(name="p2", bufs=2, space="PSUM"))

        # DMA issue order == device consumption order; every transfer is one
        # large contiguous copy (host pre-packed).  Pass-0 xt and w1 group 0
        # arrive as d-stripe halves so the first accumulation steps
        # (dd 0..3) start after ~1.1MB instead of 2.2MB.
        DH = DT // 2
        xt0_h, w1g0_h = [], []
        for half in range(2):
            t = const.tile([P, DH * WG], mybir.dt.bfloat16, tag=f"w1g0_{half}")
            nc.sync.dma_start(t[:], w1[0, :, half * DH * WG:(half + 1) * DH * WG])
            w1g0_h.append(t)
            t = const.tile([P, DH * XP], mybir.dt.bfloat16, tag=f"xt0_{half}")
            nc.sync.dma_start(t[:], xt[0, :, half * DH * XP:(half + 1) * DH * XP])
            xt0_h.append(t)
        xt_sb = [xt0_h]
        w1_sb = [w1g0_h]
        bias_sb = const.tile([P, HT + DT], mybir.dt.float32, tag="bias")
        nc.sync.dma_start(bias_sb[:], bias[:])
        b1_sb, b2_sb = bias_sb[:, :HT], bias_sb[:, HT:]
        for g in range(1, NG):
            t = const.tile([P, DT * WG], mybir.dt.bfloat16, tag=f"w1g{g}")
            nc.sync.dma_start(t[:], w1[g])
            w1_sb.append(t)
        for ps in range(1, npass):
            t = const.tile([P, DT * XP], mybir.dt.bfloat16, tag=f"xt{ps}")
            nc.sync.dma_start(t[:], xt[ps])
            xt_sb.append(t)
        w2_sb = []
        for q in range(NQ):
            t = const.tile([P, HQ * D], mybir.dt.bfloat16, tag=f"w2q{q}")
            nc.sync.dma_start(t[:], w2[q])
            w2_sb.append(t)

        # HAM warm-up: dummy matmuls during the input-DMA ramp (PE is idle
        # there) so the clock gate is at 2.4 GHz when real matmuls start.
        warm = const.tile([P, P], mybir.dt.bfloat16, tag="warm")
        nc.vector.memset(warm[:], 0)
        wp = p2.tile([P, PB], mybir.dt.float32, tag="p2")
        for _ in range(12):
            nc.tensor.matmul(wp[:, :P], warm[:], warm[:], start=True, stop=True)

        silu = mybir.ActivationFunctionType.Silu
        for ps in range(npass):
            t0 = ps * PASS
            # ---- GEMM1: HT[h, t] = silu(sum_d W1[d,h] * XT[d,t] + b1[h]) ----
            hts = []
            for hh in range(HT):
                g, ho = hh // 4, (hh % 4) * P
                pp = p1.tile([P, PB], mybir.dt.float32, tag="p1")
                for dd in range(DT):
                    if g == 0:
                        dl = dd % (DT // 2)
                        lhsT = w1_sb[0][dd // (DT // 2)][:, dl * WG + ho:dl * WG + ho + P]
                    else:
                        lhsT = w1_sb[g][:, dd * WG + ho:dd * WG + ho + P]
                    if ps == 0:
                        rhs_t = xt_sb[0][dd // (DT // 2)]
                        off = (dd % (DT // 2)) * XP
                    else:
                        rhs_t = xt_sb[ps]
                        off = dd * XP
                    for so, po, cw in cks:
                        nc.tensor.matmul(pp[:, po:po + cw], lhsT,
                                         rhs_t[:, off + so:off + so + cw],
                                         start=(dd == 0), stop=(dd == DT - 1))
                ht_t = htp.tile([P, PASS], mybir.dt.bfloat16, tag="ht")
                for so, po, cw in cks:
                    nc.scalar.activation(ht_t[:, so:so + cw], pp[:, po:po + cw],
                                         silu, bias=b1_sb[:, hh:hh + 1])
                hts.append(ht_t)
            # ---- GEMM2: OUT[d, t] = sum_h W2[h,d] * HT[h,t] + b2[d] ----
            for dd in range(DT):
                pp = p2.tile([P, PB], mybir.dt.float32, tag="p2")
                for hh in range(HT):
                    lhsT = w2_sb[hh // HQ][:, (hh % HQ) * D + dd * P:(hh % HQ) * D + (dd + 1) * P]
                    for so, po, cw in cks:
                        nc.tensor.matmul(pp[:, po:po + cw], lhsT, hts[hh][:, so:so + cw],
                                         start=(hh == 0), stop=(hh == HT - 1))
                o_t = outp.tile([P, PASS], mybir.dt.float32, tag="o")
                for so, po, cw in cks:
                    nc.vector.tensor_scalar_add(o_t[:, so:so + cw], pp[:, po:po + cw],
                                                b2_sb[:, dd:dd + 1])
                nc.sync.dma_start(ot_r[dd][:, t0:t0 + PASS], o_t[:])
    nc.compile()
    return nc


def kernel(x, Wg, W1, b1, W2, b2):
    global last_results
    x = np.asarray(x, dtype=np.float32)
    Wg = np.asarray(Wg, dtype=np.float32)
    W1 = np.asarray(W1, dtype=np.float32)
    b1 = np.asarray(b1, dtype=np.float32)
    W2 = np.asarray(W2, dtype=np.float32)
    b2 = np.asarray(b2, dtype=np.float32)
    xf = x.reshape(T, D)

    # ---- router (exact fp32, mirrors reference) ----
    logits = xf @ Wg.T
    mx = logits.max(-1, keepdims=True)
    ex = np.exp(logits - mx)
    probs = ex / ex.sum(-1, keepdims=True)
    top_i = np.argsort(-probs, axis=-1, kind="stable")[:, :TOP_K]
    top_v = np.take_along_axis(probs, top_i, axis=-1)
    top_v = top_v / top_v.sum(-1, keepdims=True)

    # aux load-balancing loss (exact)
    f_ = np.zeros(E, np.float64)
    for k in range(TOP_K):
        np.add.at(f_, top_i[:, k], 1.0)
    f_ /= T
    p_ = probs.mean(0, dtype=np.float64)
    aux_loss = np.asarray(E * (f_ * p_).sum(), dtype=np.float32)

    # ---- group tokens by expert ----
    sels, gates = [], []
    maxcnt = 0
    for e in range(E):
        sel = np.where((top_i == e).any(axis=1))[0]
        g = np.where(top_i[sel, 0] == e, top_v[sel, 0], top_v[sel, 1]).astype(np.float32)
        sels.append(sel)
        gates.append(g)
        maxcnt = max(maxcnt, len(sel))
    C, npass, PASS = _plan(maxcnt)

    key = (C, npass, PASS)
    if key not in _cache:
        _cache[key] = _build(C, npass, PASS)
    nc = _cache[key]

    XP = _xpad(PASS)
    in_maps = []
    for e in range(E):
        sel = sels[e]
        Xp = np.zeros((C, D), BF16)
        Xp[:len(sel)] = xf[sel].astype(BF16)
        # [npass, P, DT*XP]; [ps, p, dd*XP+c] = X[ps*PASS+c, dd*128+p]
        xtp = np.zeros((npass, P, DT, XP), BF16)
        xtp[:, :, :, :PASS] = Xp.reshape(npass, PASS, DT, P).transpose(0, 3, 2, 1)
        xtp = xtp.reshape(npass, P, DT * XP)
        # [NG, P, DT*WG]; [g, p, dd*WG+c] = W1[dd*128+p, g*WG+c]
        w1p = np.ascontiguousarray(
            W1[e].astype(BF16).reshape(DT, P, NG, WG).transpose(2, 1, 0, 3)
        ).reshape(NG, P, DT * WG)
        # [NQ, P, HQ*D]; [q, p, hi*D+c] = W2[(q*HQ+hi)*128+p, c]
        w2p = np.ascontiguousarray(
            W2[e].astype(BF16).reshape(NQ, HQ, P, D).transpose(0, 2, 1, 3)
        ).reshape(NQ, P, HQ * D)
        biasp = np.concatenate(
            [b1[e].reshape(HT, P).T, b2[e].reshape(DT, P).T], axis=1
        ).astype(np.float32)
        in_maps.append({
            "xt": xtp,
            "w1": w1p,
            "bias": np.ascontiguousarray(biasp),
            "w2": w2p,
        })

    trace = os.environ.get("MOE_TRACE", "0") == "1"
    last_results = run_bass_kernel_spmd(nc, in_maps, core_ids=list(range(E)), trace=trace)

    # ---- weighted combine (scatter-add) ----
    out = np.zeros((T, D), np.float32)
    for e in range(E):
        sel = sels[e]
        y = last_results.results[e]["outt"][:, :len(sel)].T  # (n_e, D)
        out[sel] += gates[e][:, None] * y

    return out.reshape(B, S, D), aux_loss


# revision 27
# speedup vs baseline: 1.0117x; 1.0117x over previous
"""MoE layer (E=8 experts, top-2 routing) on 8 trn2 NeuronCores.

Strategy
--------
The reference computes the FFN densely (every token through every expert)
and then combines with a (T, E) weight matrix that is zero outside each
token's top-2 experts.  Only the sparse grouped-GEMM is needed:

  host   : router (tiny fp32 GEMM + softmax + top-2), group tokens by
           expert, pad each group to C tokens, transpose to (D, C).
  device : core e runs expert e's MLP on its token group:
              OUT^T = W2^T-contract( silu(W1^T-contract(X^T) + b1) ) + b2
           in transposed-activation layout (contraction dim on SBUF
           partitions) so no on-device transposes are needed.  bf16
           operands, fp32 PSUM accumulation.  Weights/activations are
           pre-packed on host so every DMA is one large fully-contiguous
           transfer in device consumption order.
  host   : scale rows by the gate weights and scatter-add back; aux loss
           computed exactly on host.
"""

import os
import numpy as np
import ml_dtypes
from contextlib import ExitStack

import concourse.bass as bass
import concourse.tile as tile
from concourse import bacc, mybir
from concourse.bass_utils import run_bass_kernel_spmd

BF16 = ml_dtypes.bfloat16

B, S, D, H, E, TOP_K = 2, 2048, 1024, 4096, 8, 2
T = B * S
P = 128
DT, HT = D // P, H // P       # 8 d-tiles, 32 h-tiles
WG = 512                      # w1 column-group width (4 h-tiles per group)
NG = H // WG                  # 8 w1 groups
NQ = 4                        # w2 quarters
HQ = HT // NQ                 # 8 h-tiles per w2 quarter

_cache: dict = {}
last_results = None           # BassKernelResults of the most recent run (for test.py)


def _plan(maxcnt: int):
    """Pick padded token count C and pass split (tokens processed per pass).

    PASS <= 512 so each (h,d) step is a single matmul into one PSUM bank;
    npass minimal so per-instruction overhead is minimal."""
    c = max(2, (maxcnt + 1) // 2 * 2)
    npass = -(-c // 512)
    ps = -(-c // npass)
    ps = (ps + 1) // 2 * 2
    return ps * npass, npass, ps


def _chunks(ps: int):
    """(sbuf_off, psum_off, width) sub-chunks of one pass.

    Two balanced chunks (each >128 cols so one MM's stream covers the
    previous MM's PE drain); second chunk starts at PSUM column 512 so each
    matmul output stays inside a single 2KB PSUM bank."""
    if ps <= 512:
        return [(0, 0, ps)]
    ca = (ps + 1) // 2
    return [(0, 0, ca), (ca, 512, ps - ca)]


def _xpad(ps: int):
    """xt row stride (elements): pass width padded to a 64-element multiple
    so each d-stripe's slice starts 128B-aligned in SBUF."""
    return (ps + 63) // 64 * 64


def _build(C: int, npass: int, PASS: int):
    """Build + compile the SPMD per-expert MLP program."""
    XP = _xpad(PASS)
    nc = bacc.Bacc("TRN2", target_bir_lowering=False, debug=False, num_devices=E)
    xt = nc.dram_tensor("xt", [npass, P, DT * XP], mybir.dt.bfloat16, kind="ExternalInput").ap()
    w1 = nc.dram_tensor("w1", [NG, P, DT * WG], mybir.dt.bfloat16, kind="ExternalInput").ap()
    bias = nc.dram_tensor("bias", [P, HT + DT], mybir.dt.float32, kind="ExternalInput").ap()
    w2 = nc.dram_tensor("w2", [NQ, P, HQ * D], mybir.dt.bfloat16, kind="ExternalInput").ap()
    ot = nc.dram_tensor("outt", [D, C], mybir.dt.float32, kind="ExternalOutput").ap()
    ot_r = ot.rearrange("(a p) n -> a p n", p=P)

    cks = _chunks(PASS)
    PB = 1024 if len(cks) > 1 else 512   # psum tile width (1 or 2 banks)

    with tile.TileContext(nc) as tc, ExitStack() as ctx:
        const = ctx.enter_context(tc.tile_pool(name="const", bufs=1))
        htp = ctx.enter_context(tc.tile_pool(name="ht", bufs=HT))
        outp = ctx.enter_context(tc.tile_pool(name="out", bufs=2))
        p1 = ctx.enter_context(tc.tile_pool(name="p1", bufs=6 if PB == 512 else 2, space="PSUM"))
        p2 = ctx.enter_context(tc.tile_pool(name="p2", bufs=2, space="PSUM"))

        # DMA issue order == device consumption order; every transfer is one
        # large contiguous copy (host pre-packed).  Pass-0 xt and w1 group 0
        # arrive as d-stripe halves so the first accumulation steps
        # (dd 0..3) start after ~1.1MB instead of 2.2MB.
        DH = DT // 2
        xt0_h, w1g0_h = [], []
        for half in range(2):
            t = const.tile([P, DH * WG], mybir.dt.bfloat16, tag=f"w1g0_{half}")
            nc.sync.dma_start(t[:], w1[0, :, half * DH * WG:(half + 1) * DH * WG])
            w1g0_h.append(t)
            t = const.tile([P, DH * XP], mybir.dt.bfloat16, tag=f"xt0_{half}")
            nc.sync.dma_start(t[:], xt[0, :, half * DH * XP:(half + 1) * DH * XP])
            xt0_h.append(t)
        xt_sb = [xt0_h]
        w1_sb = [w1g0_h]
        bias_sb = const.tile([P, HT + DT], mybir.dt.float32, tag="bias")
        nc.sync.dma_start(bias_sb[:], bias[:])
        b1_sb, b2_sb = bias_sb[:, :HT], bias_sb[:, HT:]
        for g in range(1, NG):
            t = const.tile([P, DT * WG], mybir.dt.bfloat16, tag=f"w1g{g}")
            nc.sync.dma_start(t[:], w1[g])
            w1_sb.append(t)
        for ps in range(1, npass):
            t = const.tile([P, DT * XP], mybir.dt.bfloat16, tag=f"xt{ps}")
            nc.sync.dma_start(t[:], xt[ps])
            xt_sb.append(t)
        w2_sb = []
        for q in range(NQ):
            t = const.tile([P, HQ * D], mybir.dt.bfloat16, tag=f"w2q{q}")
            nc.sync.dma_start(t[:], w2[q])
            w2_sb.append(t)

        # HAM warm-up: dummy matmuls spanning the whole input-DMA ramp (PE is
        # idle there) so the clock gate reaches and HOLDS 2.4 GHz by the time
        # real matmuls start.  Also fire one dummy Silu so the ACT LUT loads
        # now instead of lazily before the first real activation.
        warm = const.tile([P, 360], mybir.dt.bfloat16, tag="warm")
        nc.vector.memset(warm[:], 0)
        wact = const.tile([P, 1], mybir.dt.float32, tag="wact")
        nc.scalar.activation(wact[:], warm[:, :1], mybir.ActivationFunctionType.Silu,
                             bias=0.0)
        wp = p2.tile([P, PB], mybir.dt.float32, tag="p2")
        for _ in range(16):
            nc.tensor.matmul(wp[:, :360], warm[:, :P], warm[:], start=True, stop=True)

        silu = mybir.ActivationFunctionType.Silu
        for ps in range(npass):
            t0 = ps * PASS
            # ---- GEMM1: HT[h, t] = silu(sum_d W1[d,h] * XT[d,t] + b1[h]) ----
            hts = []
            for hh in range(HT):
                g, ho = hh // 4, (hh % 4) * P
                pp = p1.tile([P, PB], mybir.dt.float32, tag="p1")
                for dd in range(DT):
                    if g == 0:
                        dl = dd % (DT // 2)
                        lhsT = w1_sb[0][dd // (DT // 2)][:, dl * WG + ho:dl * WG + ho + P]
                    else:
                        lhsT = w1_sb[g][:, dd * WG + ho:dd * WG + ho + P]
                    if ps == 0:
                        rhs_t = xt_sb[0][dd // (DT // 2)]
                        off = (dd % (DT // 2)) * XP
                    else:
                        rhs_t = xt_sb[ps]
                        off = dd * XP
                    for so, po, cw in cks:
                        nc.tensor.matmul(pp[:, po:po + cw], lhsT,
                                         rhs_t[:, off + so:off + so + cw],
                                         start=(dd == 0), stop=(dd == DT - 1))
                ht_t = htp.tile([P, PASS], mybir.dt.bfloat16, tag="ht")
                for so, po, cw in cks:
                    nc.scalar.activation(ht_t[:, so:so + cw], pp[:, po:po + cw],
                                         silu, bias=b1_sb[:, hh:hh + 1])
                hts.append(ht_t)
            # ---- GEMM2: OUT[d, t] = sum_h W2[h,d] * HT[h,t] + b2[d] ----
            for dd in range(DT):
                pp = p2.tile([P, PB], mybir.dt.float32, tag="p2")
                for hh in range(HT):
                    lhsT = w2_sb[hh // HQ][:, (hh % HQ) * D + dd * P:(hh % HQ) * D + (dd + 1) * P]
                    for so, po, cw in cks:
                        nc.tensor.matmul(pp[:, po:po + cw], lhsT, hts[hh][:, so:so + cw],
                                         start=(hh == 0), stop=(hh == HT - 1))
                o_t = outp.tile([P, PASS], mybir.dt.float32, tag="o")
                for so, po, cw in cks:
                    nc.vector.tensor_scalar_add(o_t[:, so:so + cw], pp[:, po:po + cw],
                                                b2_sb[:, dd:dd + 1])
                nc.sync.dma_start(ot_r[dd][:, t0:t0 + PASS], o_t[:])
    nc.compile()
    return nc


def kernel(x, Wg, W1, b1, W2, b2):
    global last_results
    x = np.asarray(x, dtype=np.float32)
    Wg = np.asarray(Wg, dtype=np.float32)
    W1 = np.asarray(W1, dtype=np.float32)
    b1 = np.asarray(b1, dtype=np.float32)
    W2 = np.asarray(W2, dtype=np.float32)
    b2 = np.asarray(b2, dtype=np.float32)
    xf = x.reshape(T, D)

    # ---- router (exact fp32, mirrors reference) ----
    logits = xf @ Wg.T
    mx = logits.max(-1, keepdims=True)
    ex = np.exp(logits - mx)
    probs = ex / ex.sum(-1, keepdims=True)
    top_i = np.argsort(-probs, axis=-1, kind="stable")[:, :TOP_K]
    top_v = np.take_along_axis(probs, top_i, axis=-1)
    top_v = top_v / top_v.sum(-1, keepdims=True)

    # aux load-balancing loss (exact)
    f_ = np.zeros(E, np.float64)
    for k in range(TOP_K):
        np.add.at(f_, top_i[:, k], 1.0)
    f_ /= T
    p_ = probs.mean(0, dtype=np.float64)
    aux_loss = np.asarray(E * (f_ * p_).sum(), dtype=np.float32)

    # ---- group tokens by expert ----
    sels, gates = [], []
    maxcnt = 0
    for e in range(E):
        sel = np.where((top_i == e).any(axis=1))[0]
        g = np.where(top_i[sel, 0] == e, top_v[sel, 0], top_v[sel, 1]).astype(np.float32)
        sels.append(sel)
        gates.append(g)
        maxcnt = max(maxcnt, len(sel))
    C, npass, PASS = _plan(maxcnt)

    key = (C, npass, PASS)
    if key not in _cache:
        _cache[key] = _build(C, npass, PASS)
    nc = _cache[key]

    XP = _xpad(PASS)
    in_maps = []
    for e in range(E):
        sel = sels[e]
        Xp = np.zeros((C, D), BF16)
        Xp[:len(sel)] = xf[sel].astype(BF16)
        # [npass, P, DT*XP]; [ps, p, dd*XP+c] = X[ps*PASS+c, dd*128+p]
        xtp = np.zeros((npass, P, DT, XP), BF16)
        xtp[:, :, :, :PASS] = Xp.reshape(npass, PASS, DT, P).transpose(0, 3, 2, 1)
        xtp = xtp.reshape(npass, P, DT * XP)
        # [NG, P, DT*WG]; [g, p, dd*WG+c] = W1[dd*128+p, g*WG+c]
        w1p = np.ascontiguousarray(
            W1[e].astype(BF16).reshape(DT, P, NG, WG).transpose(2, 1, 0, 3)
        ).reshape(NG, P, DT * WG)
        # [NQ, P, HQ*D]; [q, p, hi*D+c] = W2[(q*HQ+hi)*128+p, c]
        w2p = np.ascontiguousarray(
            W2[e].astype(BF16).reshape(NQ, HQ, P, D).transpose(0, 2, 1, 3)
        ).reshape(NQ, P, HQ * D)
        biasp = np.concatenate(
            [b1[e].reshape(HT, P).T, b2[e].reshape(DT, P).T], axis=1
        ).astype(np.float32)
        in_maps.append({
            "xt": xtp,
            "w1": w1p,
            "bias": np.ascontiguousarray(biasp),
            "w2": w2p,
        })

    trace = os.environ.get("MOE_TRACE", "0") == "1"
    last_results = run_bass_kernel_spmd(nc, in_maps, core_ids=list(range(E)), trace=trace)

    # ---- weighted combine (scatter-add) ----
    out = np.zeros((T, D), np.float32)
    for e in range(E):
        sel = sels[e]
        y = last_results.results[e]["outt"][:, :len(sel)].T  # (n_e, D)
        out[sel] += gates[e][:, None] * y

    return out.reshape(B, S, D), aux_loss
